# revision 30
# baseline (speedup 1.0000x reference)
"""Trainium2 Bass kernel for nn_BatchedGaussianRenderer.

Math: each gaussian's per-pixel exponent is expanded as a 6-term polynomial
in centered pixel coordinates (x', y') = (x-63.5, y-63.5):

  expo(n, x, y) = f1*x'^2 + f2*x'y' + f3*y'^2 + f4*x' + f5*y' + f6
  image(x, y)   = sum_n exp(expo(n, x, y)),  then / max(image)

so the dense N x P evaluation is a K=6 matmul.  For accuracy on the bf16
tensor engine, f and the pixel basis g are each split into 2 bf16
components and the 3 leading cross products kept (K=18, error ~2^-17 per
term, validated at ~3e-6 absmax-rel vs an fp64 oracle).

Sharding: each of the 8 cores computes ALL 4096 gaussians' coefficients
(cheap, ~100 vector ops on [128, blocks] layouts) and renders 16 image
rows (pixels x on partitions, gaussians streamed).  K=18 <= 32 lets the
dense matmuls be packed 4-to-the-PE-array via tile_position row groups:
per image row, 4 concurrent [18x128]x[18x512] matmuls cover 2048
gaussians in ~512 PE cycles.  The ScalarEngine's fused exp+row-sum
(accum_out) produces the image directly; an 8-value AllGather + local max
replaces the AllReduce for the final normalization.

The gaussians are processed in two halves so the second half's coefficient
computation (VectorEngine) overlaps the first half's rendering (ScalarE).

Per-gaussian preprocessing uses the unnormalized-quaternion fold: cov4D
scales uniformly by nsq = |q1|^2 |q2|^2, which cancels everywhere except
eps -> eps*nsq and inv_cov/lambda -> *nsq, avoiding rsqrt entirely.
sin/cos of the view angle are evaluated as Taylor polynomials on the DVE
(angle in [0,1)) so the only ACT table set ever loaded is exp's.
"""
import numpy as np
import ml_dtypes

import concourse.bass as bass
import concourse.bacc as bacc
import concourse.tile as tile
import concourse.mybir as mybir
from concourse import bass_utils

NG, H, W = 4096, 128, 128
ZOOM, EPS = 0.5, 1e-6
CX = CY = 63.5
SXY = (W - 1) / 2 * ZOOM          # 31.75
NCORES = 8
ROWS = H // NCORES                # 16 image rows per core
NB = NG // 128                    # 32 gaussian blocks (g = p*NB + b)
HB = NB // 2                      # 16 blocks per half
NSLOT = 3                         # (f-split, g-split) pairs: 00, 01, 10
KP = NSLOT * 6                    # 18 K rows
dt = mybir.dt
AF = mybir.ActivationFunctionType
ALU = mybir.AluOpType

# ---------------------------------------------------------------- host helpers

def _bf16(x):
    return np.asarray(x, np.float32).astype(ml_dtypes.bfloat16).astype(np.float32)


def _g_lhsT_for_core(core):
    """[128, ROWS*128] bf16 pixel-basis weights, replicated into the four
    32-partition groups (rows 32i..32i+17 identical) for tile_position
    row-group packing; see module docstring."""
    jg_of_s = (0, 1, 0)           # g-split component per slot
    out = np.zeros((128, ROWS * 128), np.float32)
    x = np.arange(128, dtype=np.float64) - CX
    for r in range(ROWS):
        y = ROWS * core + r - CY
        # slot order pairs the device F6 layout (ID_, IA, F2T, f5, f4, f6'):
        basis = np.stack([np.full(128, -0.5 * y * y), -0.5 * x * x, x * y,
                          np.full(128, y), x, np.full(128, -0.5)], 0)
        b32 = basis.astype(np.float32)
        g0 = _bf16(b32)
        g1 = _bf16(b32 - g0)
        gs = (g0, g1)
        for s in range(NSLOT):
            for k in range(6):
                row = gs[jg_of_s[s]][k]
                for i in range(4):
                    out[32 * i + s * 6 + k, 128 * r:128 * (r + 1)] = row
    return out.astype(ml_dtypes.bfloat16)


# L(q1) twisted copies: (out_off, out_stride, in_off, in_stride, count, sign),
# L stored per-block (i,k) slot = i*4+k, rotor comps a=(r0,r4,r5,r6).
L_COPIES = [
    (0, 1, 0, 1, 1, 1.0), (4, 4, 4, 1, 3, 1.0),
    (1, 12, 4, 1, 2, -1.0), (5, 4, 0, 6, 2, 1.0),
    (2, 4, 5, 1, 2, -1.0), (10, 4, 0, 4, 2, 1.0),
    (3, 1, 6, 1, 1, -1.0), (11, 1, 4, 1, 1, -1.0),
    (7, 1, 5, 1, 1, 1.0), (15, 1, 0, 1, 1, 1.0),
]
# R(conj q2) with q2 = (r7, -r1, -r2, -r3) folded; stored (j,k) slot = j*4+k.
R_COPIES = [
    (0, 1, 7, 1, 1, 1.0), (4, 4, 1, 1, 3, -1.0),
    (1, 4, 1, 6, 2, 1.0), (9, 1, 3, 1, 1, -1.0), (13, 1, 2, 1, 1, 1.0),
    (2, 4, 2, 1, 2, 1.0), (10, 1, 7, 1, 1, 1.0), (14, 1, 1, 1, 1, -1.0),
    (3, 1, 3, 1, 1, 1.0), (7, 1, 2, 1, 1, -1.0), (11, 4, 1, 6, 2, 1.0),
]

SIN_C = [1.0, -1.0 / 6, 1.0 / 120, -1.0 / 5040, 1.0 / 362880]      # of x^(2k+1)
COS_C = [1.0, -0.5, 1.0 / 24, -1.0 / 720, 1.0 / 40320, -1.0 / 3628800]


def build_nc():
    nc = bacc.Bacc("TRN2", target_bir_lowering=False, debug=False,
                   num_devices=NCORES)
    f32, bf16 = dt.float32, dt.bfloat16

    # fused inputs: one DMA per dtype class (9 serial DMAs cost ~6us of startup)
    fusf_in = nc.dram_tensor("fused_f32", [128, 640], f32, kind="ExternalInput").ap()
    fusb_in = nc.dram_tensor("fused_bf16", [128, ROWS * 128 + 128], bf16,
                             kind="ExternalInput").ap()
    scal_in = nc.dram_tensor("fused_scal", [1, 131], f32, kind="ExternalInput").ap()
    out_t = nc.dram_tensor("out", [ROWS, W], f32, kind="ExternalOutput").ap()

    with tile.TileContext(nc) as tc:
        with (
            tc.tile_pool(name="sb", bufs=1) as sb,
            tc.tile_pool(name="dram", bufs=1, space="DRAM") as dram,
        ):
            # ---------------- phase 0: loads + angle scalars ----------------
            FUSF = sb.tile([128, 640], f32)
            FUSB = sb.tile([128, ROWS * 128 + 128], bf16)
            SCAL = sb.tile([1, 131], f32)
            nc.sync.dma_start(FUSF[:], fusf_in[:])
            nc.sync.dma_start(SCAL[:], scal_in[:])
            nc.sync.dma_start(FUSB[:], fusb_in[:])
            MEANS = FUSF[:, 0:128]
            RAWS = FUSF[:, 128:256]
            ROT = FUSF[:, 256:512]
            IDF = FUSF[:, 512:640]
            G_SB = FUSB[:, 0:ROWS * 128]
            IDB = FUSB[:, ROWS * 128:ROWS * 128 + 128]
            T_A = SCAL[:, 0:1]
            ANG = SCAL[:, 1:2]
            ONES = SCAL[:, 3:131]

            # sin/cos via Taylor on DVE (angle in [0,1); no trig table load)
            U = sb.tile([1, 1], f32)
            SINA = sb.tile([1, 1], f32)
            COSA = sb.tile([1, 1], f32)
            nc.vector.tensor_mul(U[:], ANG, ANG)
            # cos/sin: Horner in u = x^2 (acc = acc*u + c per step)
            nc.vector.tensor_scalar(COSA[:], U[:], COS_C[5], COS_C[4],
                                    ALU.mult, ALU.add)
            for c in (COS_C[3], COS_C[2], COS_C[1], COS_C[0]):
                nc.vector.tensor_scalar(COSA[:], COSA[:], U[:], c,
                                        ALU.mult, ALU.add)
            nc.vector.tensor_scalar(SINA[:], U[:], SIN_C[4], SIN_C[3],
                                    ALU.mult, ALU.add)
            for c in (SIN_C[2], SIN_C[1], SIN_C[0]):
                nc.vector.tensor_scalar(SINA[:], SINA[:], U[:], c,
                                        ALU.mult, ALU.add)
            nc.vector.tensor_mul(SINA[:], SINA[:], ANG)

            # scalar vector, laid out so batched preprocessing ops can use
            # adjacent broadcast pairs/triples:
            # [sxc, SXY, sxs, t, A1, B1, A2, A3, S2Y, B2]
            SCV = sb.tile([1, 12], f32)
            nc.vector.tensor_scalar_mul(SCV[:, 0:1], COSA[:], float(SXY))
            nc.vector.tensor_scalar_mul(SCV[:, 1:2], ONES[:, 0:1], float(SXY))
            nc.vector.tensor_scalar_mul(SCV[:, 2:3], SINA[:], float(SXY))
            nc.vector.tensor_copy(SCV[:, 3:4], T_A)
            nc.vector.tensor_mul(SCV[:, 4:5], SCV[:, 0:1], SCV[:, 0:1])
            nc.vector.tensor_scalar_mul(SCV[:, 5:6], SCV[:, 0:1], float(SXY))
            nc.vector.scalar_tensor_tensor(SCV[:, 6:7], SCV[:, 0:1], 2.0,
                                           SCV[:, 2:3], ALU.mult, ALU.mult)
            nc.vector.tensor_mul(SCV[:, 7:8], SCV[:, 2:3], SCV[:, 2:3])
            nc.vector.tensor_scalar_mul(SCV[:, 8:9], ONES[:, 0:1],
                                        float(SXY * SXY))
            nc.vector.tensor_scalar_mul(SCV[:, 9:10], SCV[:, 2:3], float(SXY))
            with tc.tile_pool(name="pp0", bufs=1, space="PSUM") as pp0:
                PBp = pp0.tile([128, 12], f32)
                nc.tensor.matmul(PBp[:], ONES, SCV[:], start=True, stop=True)
                SCB = sb.tile([128, 12], f32)
                nc.vector.tensor_copy(SCB[:], PBp[:])
            sxs_b = SCB[:, 2:3]
            tb = SCB[:, 3:4]
            A3b = SCB[:, 7:8]

            # warm the CC channel early: the first collective in a NEFF pays
            # tens of us of one-time setup; run a junk AllGather overlapped
            # with preprocessing/dense so the real one at the tail is cheap.


            means_a = MEANS.rearrange("p (b c) -> p b c", c=4)
            rot_a = ROT.rearrange("p (b c) -> p b c", c=8)

            def preprocess_half(h):
                """Emit coefficient computation for blocks [HB*h, HB*(h+1)).
                Returns the F18 tile [128, HB*18] bf16 (b, s, k)."""
                tg = lambda n: f"{n}{h}"
                B = HB
                means_h = means_a[:, HB * h:HB * (h + 1), :]
                rot_h = rot_a[:, HB * h:HB * (h + 1), :]
                raws_h = RAWS[:, HB * 4 * h:HB * 4 * (h + 1)]

                S2 = sb.tile([128, B * 4], f32, tag=tg("S2"))
                nc.scalar.activation(S2[:], raws_h, AF.Exp, scale=2.0)

                SQ = sb.tile([128, B * 8], f32, tag=tg("SQ"))
                nc.vector.tensor_mul(SQ[:], rot_h, rot_h)
                sq = SQ[:].rearrange("p (b c) -> p b c", c=8)
                N1S = sb.tile([128, B], f32, tag=tg("N1S"))
                N2S = sb.tile([128, B], f32, tag=tg("N2S"))
                NSQ = sb.tile([128, B], f32, tag=tg("NSQ"))
                nc.vector.reduce_sum(N1S[:], sq[:, :, 4:7], axis=mybir.AxisListType.X)
                nc.vector.tensor_add(N1S[:], N1S[:], sq[:, :, 0])
                nc.vector.reduce_sum(N2S[:], sq[:, :, 1:4], axis=mybir.AxisListType.X)
                nc.vector.tensor_add(N2S[:], N2S[:], sq[:, :, 7])
                nc.vector.tensor_mul(NSQ[:], N1S[:], N2S[:])

                LT = sb.tile([128, B * 16], f32, tag=tg("LT"))
                RT = sb.tile([128, B * 16], f32, tag=tg("RT"))
                lt3 = LT[:].rearrange("p (b c) -> p b c", c=16)
                rt3 = RT[:].rearrange("p (b c) -> p b c", c=16)
                # Twisted copies split across DVE and GpSimd so neither's
                # serial chain gates the P64 product for long.
                for ci, (dst, (oo, os_, io, is_, cnt, sign)) in enumerate(
                        [(lt3, c) for c in L_COPIES] + [(rt3, c) for c in R_COPIES]):
                    eng = nc.gpsimd if ci % 3 == 2 else nc.vector
                    out_ap = dst[:, :, oo::os_][:, :, :cnt] if cnt > 1 else dst[:, :, oo:oo + 1]
                    in_ap = rot_h[:, :, io::is_][:, :, :cnt] if cnt > 1 else rot_h[:, :, io:io + 1]
                    if sign > 0:
                        eng.tensor_copy(out_ap, in_ap)
                    else:
                        eng.tensor_scalar_mul(out_ap, in_ap, -1.0)

                P64 = sb.tile([128, B * 64], f32, tag=tg("P64"))
                lt4 = LT[:].rearrange("p (b i k) -> p b i k", i=4, k=4)
                rt4 = RT[:].rearrange("p (b j k) -> p b j k", j=4, k=4)
                p5 = P64[:].rearrange("p (b i j k) -> p b i j k", i=4, j=4, k=4)
                nc.vector.tensor_mul(
                    p5,
                    lt4.unsqueeze(3).broadcast_to([128, B, 4, 4, 4]),
                    rt4.unsqueeze(2).broadcast_to([128, B, 4, 4, 4]))
                R4 = sb.tile([128, B * 16], f32, tag=tg("R4"))
                nc.vector.reduce_sum(
                    R4[:], P64[:].rearrange("p (e k) -> p e k", k=4),
                    axis=mybir.AxisListType.X)

                M = sb.tile([128, B * 16], f32, tag=tg("M"))
                r44 = R4[:].rearrange("p (b i j) -> p b i j", i=4, j=4)
                s23 = S2[:].rearrange("p (b c) -> p b c", c=4)
                m4 = M[:].rearrange("p (b i j) -> p b i j", i=4, j=4)
                nc.vector.tensor_mul(
                    m4, r44, s23.unsqueeze(2).broadcast_to([128, B, 4, 4]))
                C64 = sb.tile([128, B * 64], f32, tag=tg("C64"))
                c5 = C64[:].rearrange("p (b i k j) -> p b i k j", i=4, k=4, j=4)
                nc.vector.tensor_mul(
                    c5,
                    m4.unsqueeze(3).broadcast_to([128, B, 4, 4, 4]),
                    r44.unsqueeze(2).broadcast_to([128, B, 4, 4, 4]))
                C16 = sb.tile([128, B * 16], f32, tag=tg("C16"))
                nc.vector.reduce_sum(
                    C16[:], C64[:].rearrange("p (e j) -> p e j", j=4),
                    axis=mybir.AxisListType.X)
                c16 = C16[:].rearrange("p (b e) -> p b e", e=16)

                EPN = sb.tile([128, B], f32, tag=tg("EPN"))
                nc.vector.tensor_scalar_mul(EPN[:], NSQ[:], float(EPS))
                WP = sb.tile([128, B], f32, tag=tg("WP"))
                nc.vector.tensor_max(WP[:], c16[:, :, 15], EPN[:])
                IW = sb.tile([128, B], f32, tag=tg("IW"))
                nc.vector.reciprocal(IW[:], WP[:])
                TD = sb.tile([128, B], f32, tag=tg("TD"))
                nc.scalar.activation(TD[:], means_h[:, :, 3], AF.Identity,
                                     bias=tb, scale=-1.0)
                TDW = sb.tile([128, B], f32, tag=tg("TDW"))
                nc.vector.tensor_mul(TDW[:], TD[:], IW[:])
                W1 = sb.tile([128, B], f32, tag=tg("W1"))
                nc.vector.tensor_mul(W1[:], NSQ[:], IW[:])
                Z3 = sb.tile([128, B], f32, tag=tg("Z3"))
                nc.vector.tensor_mul(Z3[:], W1[:], TD[:])

                VV9 = sb.tile([128, B * 9], f32, tag=tg("VV9"))
                vv3 = VV9[:].rearrange("p (b i k) -> p b i k", i=3, k=3)
                v_i = c16[:, :, 3::4][:, :, 0:3]
                nc.vector.tensor_mul(
                    vv3,
                    v_i.unsqueeze(3).broadcast_to([128, B, 3, 3]),
                    v_i.unsqueeze(2).broadcast_to([128, B, 3, 3]))
                CV3 = sb.tile([128, B * 9], f32, tag=tg("CV3"))
                cv3f = CV3[:].rearrange("p (b e) -> p b e", e=9)
                iw_b9 = IW[:].unsqueeze(2).broadcast_to([128, B, 9])
                vv9f = VV9[:].rearrange("p (b e) -> p b e", e=9)
                nc.vector.tensor_mul(cv3f, vv9f, iw_b9)
                u9 = c16.rearrange("p b (i k) -> p b i k", i=4)[:, :, 0:3, 0:3]
                cv33 = CV3[:].rearrange("p (b i k) -> p b i k", i=3, k=3)
                nc.vector.tensor_sub(cv33, u9, cv33)

                MU3 = sb.tile([128, B * 3], f32, tag=tg("MU3"))
                mu33 = MU3[:].rearrange("p (b c) -> p b c", c=3)
                tdw_b3 = TDW[:].unsqueeze(2).broadcast_to([128, B, 3])
                nc.vector.tensor_mul(mu33, v_i, tdw_b3)
                nc.vector.tensor_add(mu33, mu33, means_h[:, :, 0:3])

                # Batched coefficient finish.  Slot order (matched by the
                # host g basis): f = (ID_, IA, F2T, f5, f4, f6') pairing
                # g = (-y^2/2, -x^2/2, xy, y, x, -1/2).
                TMP = sb.tile([128, B], f32, tag=tg("TMP"))
                T2 = sb.tile([128, B * 2], f32, tag=tg("T2"))    # (MX, MY)
                t23 = T2[:].rearrange("p (b c) -> p b c", c=2)
                nc.vector.tensor_mul(
                    t23, mu33[:, :, 0:2],
                    SCB[:, 0:2].unsqueeze(1).broadcast_to([128, B, 2]))
                nc.vector.scalar_tensor_tensor(t23[:, :, 0], mu33[:, :, 2],
                                               sxs_b, t23[:, :, 0],
                                               ALU.mult, ALU.add)
                T4 = sb.tile([128, B * 2], f32, tag=tg("T4"))    # (MY, MX)
                t43 = T4[:].rearrange("p (b c) -> p b c", c=2)
                nc.vector.tensor_copy(t43[:, :, 0], t23[:, :, 1])
                nc.vector.tensor_copy(t43[:, :, 1], t23[:, :, 0])

                cv3e = CV3[:].rearrange("p (b e) -> p b e", e=9)
                MUL1 = sb.tile([128, B * 3], f32, tag=tg("MUL1"))
                m13 = MUL1[:].rearrange("p (b c) -> p b c", c=3)
                nc.vector.tensor_mul(
                    m13, cv3e[:, :, 0:3],
                    SCB[:, 4:7].unsqueeze(1).broadcast_to([128, B, 3]))
                TRI = sb.tile([128, B * 3], f32, tag=tg("TRI"))  # (AE, DE, BE)
                tri = TRI[:].rearrange("p (b c) -> p b c", c=3)
                nc.vector.tensor_mul(
                    tri[:, :, 1:3], cv3e[:, :, 4:6],
                    SCB[:, 8:10].unsqueeze(1).broadcast_to([128, B, 2]))
                nc.vector.scalar_tensor_tensor(tri[:, :, 0], cv3e[:, :, 8], A3b,
                                               m13[:, :, 0], ALU.mult, ALU.add)
                nc.vector.tensor_add(tri[:, :, 0], tri[:, :, 0], m13[:, :, 2])
                nc.vector.tensor_add(tri[:, :, 2], tri[:, :, 2], m13[:, :, 1])
                nc.vector.tensor_add(
                    tri[:, :, 0:2], tri[:, :, 0:2],
                    EPN[:].unsqueeze(2).broadcast_to([128, B, 2]))

                P2d = sb.tile([128, B * 2], f32, tag=tg("P2d"))
                p2d = P2d[:].rearrange("p (b c) -> p b c", c=2)
                nc.vector.tensor_mul(p2d, tri[:, :, 0:3:2], tri[:, :, 1:3])
                DET = sb.tile([128, B], f32, tag=tg("DET"))
                nc.vector.tensor_sub(DET[:], p2d[:, :, 0], p2d[:, :, 1])
                IDN = sb.tile([128, B], f32, tag=tg("IDN"))
                nc.vector.reciprocal(IDN[:], DET[:])
                nc.vector.tensor_mul(IDN[:], IDN[:], NSQ[:])

                F6 = sb.tile([128, B * 6], f32, tag=tg("F6"))
                f63 = F6[:].rearrange("p (b k) -> p b k", k=6)
                # (ID_, IA, F2T) = (AE, DE, BE) * IDN
                nc.vector.tensor_mul(
                    f63[:, :, 0:3], tri,
                    IDN[:].unsqueeze(2).broadcast_to([128, B, 3]))
                # (f5, f4) = (ID_*MY, IA*MX) - (F2T*MX, F2T*MY)... see below
                nc.vector.tensor_mul(p2d, f63[:, :, 0:2], t43)
                PN = sb.tile([128, B * 2], f32, tag=tg("PN"))
                pn = PN[:].rearrange("p (b c) -> p b c", c=2)
                nc.vector.tensor_mul(
                    pn, f63[:, :, 2:3].broadcast_to([128, B, 2]), t23)
                nc.vector.tensor_sub(f63[:, :, 3:5], p2d, pn)
                # f6' = MX*f4 + MY*f5 + Z3*TD   (g pairs it with -1/2)
                nc.vector.tensor_mul(pn, t43, f63[:, :, 3:5])
                nc.vector.tensor_add(f63[:, :, 5], pn[:, :, 0], pn[:, :, 1])
                nc.vector.tensor_mul(TMP[:], Z3[:], TD[:])
                nc.vector.tensor_add(f63[:, :, 5], f63[:, :, 5], TMP[:])

                # F18 bf16 slots: s0 = f0, s1 = f0 (pairs with g1), s2 = f1
                F18 = sb.tile([128, B * KP], bf16, tag=tg("F18"))
                f364 = F18[:].rearrange("p (b s k) -> p b s k", s=NSLOT, k=6)
                R1 = sb.tile([128, B * 6], f32, tag=tg("R1"))
                r13 = R1[:].rearrange("p (b k) -> p b k", k=6)
                nc.vector.tensor_copy(f364[:, :, 0, :], f63)
                nc.vector.tensor_copy(f364[:, :, 1, :], f364[:, :, 0, :])
                nc.vector.tensor_sub(r13, f63, f364[:, :, 0, :])
                nc.vector.tensor_copy(f364[:, :, 2, :], r13)
                return F18

            F18s = [None, None]
            F18s[0] = preprocess_half(0)
            # FS[h]: [128, 512] bf16; partition group i rows 32i..32i+17 hold
            # F^T for the half's i-th 4-block chunk (512 gaussians).
            FS = [sb.tile([128, 512], bf16, tag=f"FS{h}", name=f"FS{h}")
                  for h in range(2)]

            ACC = sb.tile([128, 2 * ROWS], f32)
            IMG = sb.tile([128, ROWS], f32)
            RMX = sb.tile([128, 1], f32)

            def transpose_half(h):
                TP = dp.tile([128, 512], bf16, tag="pt")
                for i in range(4):
                    for c in range(4):
                        b = 4 * i + c
                        nc.tensor.transpose(
                            TP[32 * i:32 * i + KP, 128 * c:128 * (c + 1)],
                            F18s[h][:, KP * b:KP * (b + 1)], IDB,
                            tile_position=(0, 32 * i))
                nc.vector.tensor_copy(FS[h][:], TP[:])

            def dense_sweep(r, h):
                PT = dp.tile([128, 2048], dt.float32, tag="pt")
                for i in range(4):
                    nc.tensor.matmul(
                        PT[:, 512 * i:512 * (i + 1)],
                        G_SB[32 * i:32 * i + KP, 128 * r:128 * (r + 1)],
                        FS[h][32 * i:32 * i + KP, :],
                        start=True, stop=True,
                        tile_position=(32 * i, 0))
                col = 2 * r + h
                nc.scalar.activation(PT[:], PT[:], AF.Exp,
                                     accum_out=ACC[:, col:col + 1])

            with tc.tile_pool(name="dp", bufs=2, space="PSUM") as dp:
                transpose_half(0)
                for r in range(ROWS):
                    dense_sweep(r, 0)
                    if r == 0:
                        # h1 preprocessing overlaps the h0 dense phase; the
                        # lowered priority keeps the Tile scheduler from
                        # interleaving it into h0's critical DVE chain.
                        with tc.high_priority(offset=-1_000_000):
                            F18s[1] = preprocess_half(1)
                    if r == 12:
                        transpose_half(1)
                for r in range(ROWS):
                    dense_sweep(r, 1)
                    if r == 3:
                        # warm the CC channel: the collective entry barrier and
                        # channel setup (tens of us, async on TOPSP) complete
                        # during the dense phase, and an AllGather runs shortly
                        # before the real one so the tail AG hits its ~5us
                        # hot-path floor instead of ~22us.
                        cinw = dram.tile([1, 1], f32)
                        coutw = dram.tile([NCORES, 1], f32)
                        nc.sync.dma_start(cinw[:], ACC[0:1, 7:8])
                        nc.gpsimd.collective_compute(
                            "AllGather", ALU.bypass,
                            replica_groups=[list(range(NCORES))],
                            ins=[cinw[:].opt()], outs=[coutw[:].opt()])

            acc3 = ACC[:].rearrange("p (r h) -> p r h", h=2)
            nc.vector.tensor_add(IMG[:], acc3[:, :, 0], acc3[:, :, 1])
            nc.vector.reduce_max(RMX[:], IMG[:], axis=mybir.AxisListType.X)

            # ------- phase 3: global max (AllGather) + normalize -------
            with tc.tile_pool(name="tp", bufs=1, space="PSUM") as tp:
                RMTp = tp.tile([1, 128], dt.float32)
                nc.tensor.transpose(RMTp[:], RMX[:], IDF)
                LMAX = sb.tile([1, 1], dt.float32)
                nc.vector.reduce_max(LMAX[:], RMTp[:], axis=mybir.AxisListType.X)
                cin = dram.tile([1, 1], dt.float32)
                cout = dram.tile([NCORES, 1], dt.float32)
                nc.sync.dma_start(cin[:], LMAX[:])
                nc.gpsimd.collective_compute(
                    "AllGather", ALU.bypass,
                    replica_groups=[list(range(NCORES))],
                    ins=[cin[:].opt()], outs=[cout[:].opt()])

                # transpose the unnormalized image while the collective runs
                OTp = tp.tile([ROWS, 128], dt.float32)
                nc.tensor.transpose(OTp[:], IMG[:], IDF)
                OT = sb.tile([ROWS, 128], dt.float32)
                nc.vector.tensor_copy(OT[:], OTp[:])
                GM8 = sb.tile([1, NCORES], dt.float32)
                nc.sync.dma_start(GM8[:], cout[:].rearrange("p q -> q p"))
                GM = sb.tile([1, 1], dt.float32)
                nc.vector.reduce_max(GM[:], GM8[:], axis=mybir.AxisListType.X)
                nc.vector.tensor_scalar_max(GM[:], GM[:], float(EPS))
                RI = sb.tile([1, 1], dt.float32)
                nc.vector.reciprocal(RI[:], GM[:])
                RIBp = tp.tile([ROWS, 1], dt.float32)
                nc.tensor.matmul(RIBp[:], ONES[:, 0:ROWS], RI[:],
                                 start=True, stop=True)
                RIB = sb.tile([ROWS, 1], dt.float32)
                nc.vector.tensor_copy(RIB[:], RIBp[:])
                nc.vector.tensor_scalar(OT[:], OT[:], RIB[:], None, ALU.mult)
                nc.sync.dma_start(out_t[:], OT[:])

    nc.compile()
    return nc


_NC_CACHE = {}


def _get_nc():
    if "nc" not in _NC_CACHE:
        _NC_CACHE["nc"] = build_nc()
    return _NC_CACHE["nc"]


def _make_in_maps(means, raw_scales, rotors, t, angle):
    means = np.asarray(means, np.float32).reshape(128, 128)
    raw_scales = np.asarray(raw_scales, np.float32).reshape(128, 128)
    rotors = np.asarray(rotors, np.float32).reshape(128, 256)
    fusf = np.concatenate(
        [means, raw_scales, rotors, np.eye(128, dtype=np.float32)], axis=1)
    fusf = np.ascontiguousarray(fusf)
    idb = np.eye(128, dtype=np.float32).astype(ml_dtypes.bfloat16)
    scal = np.ones((1, 131), np.float32)
    scal[0, 0] = np.float32(t)
    scal[0, 1] = np.float32(angle)
    in_maps = []
    for c in range(NCORES):
        fusb = np.ascontiguousarray(
            np.concatenate([_g_lhsT_for_core(c), idb], axis=1))
        in_maps.append({
            "fused_f32": fusf, "fused_bf16": fusb, "fused_scal": scal,
        })
    return in_maps


def run(means, raw_scales, rotors, t, angle, trace=False):
    """Returns (image [128,128] fp32, BassKernelResults)."""
    nc = _get_nc()
    in_maps = _make_in_maps(means, raw_scales, rotors, t, angle)
    res = bass_utils.run_bass_kernel_spmd(
        nc, in_maps, core_ids=list(range(NCORES)), trace=trace)
    img = np.concatenate([res.results[c]["out"] for c in range(NCORES)], axis=0)
    return img.astype(np.float32), res


def kernel(**inputs):
    img, _ = run(inputs["means"], inputs["raw_scales"], inputs["rotors"],
                 inputs["t"], inputs["angle"])
    return img
